# revision 5
# baseline (speedup 1.0000x reference)
"""Per-pixel adaptive 5x5 conv (KPN) for Trainium2, 8-core data parallel.

out[g,h,w] = sum_{i,j} core[g,5i+j,h,w] * frames_pad[g,h+i-2,w+j-2]
with g = flattened (B,N) = 16 image planes; 2 planes per NeuronCore.

v3 design — PE-accumulate, DVE-multiply, no GpSimd compute:
  Row layout: partition p owns output rows 4p..4p+3; fin stores the 8-row
  halo (4p-2..4p+5) x 516 padded cols per partition (x2 replication only).
  Weights are host-shifted into a 516-wide y-grid per tap (w at y=x+j) so
  every DVE read starts 4B-aligned. All 5 taps of group tg share i=tg, so
  ONE fp16 tensor_mul per group computes all 5 products (frames broadcast
  along the tap dim with a stride-0 AP):
     q[p,k,r,y] = w[p,k,r,y] * f[p,r+tg,y]
  The IDLE TensorEngine sums the 25 taps per output row r into PSUM (fp32)
  as identity-stationary matmuls, moving slice [2,512] at offset r*516+j
  (PE needs no alignment). First group of img0 runs per-tap (small DMAs +
  muls) to cut the startup ramp; warmup matmuls during the ramp release
  the PE HAM clock throttle. ACT evacuates PSUM with the fp32->fp16 cast.
  fin/out ride the second HWDGE ring (nc.scalar), weights on nc.sync.
  GpSimd does nothing: its SBUF port contends with DVE tensor_tensor.
"""

import os
import sys

import numpy as np

for _p in ("/opt/trn_rl_repo",):
    if _p not in sys.path and os.path.isdir(_p):
        sys.path.insert(0, _p)

K = 5
NCORES = 8
IMGS_PER_CORE = 2
H = W = 512
RPP = 4            # output rows per partition
FROWS = 8          # stored halo rows per partition
YC = 516           # padded column grid
F_FREE = FROWS * YC          # 4128
T_FREE = RPP * YC            # 2064 (one tap)
WG_FREE = K * T_FREE         # 10320 (one 5-tap group)
O_FREE = RPP * W             # 2048
N_WARMUP = 12

_compiled = {}
last_results = None  # BassKernelResults of the most recent run (for test.py)


def _build_nc():
    import concourse.bacc as bacc
    import concourse.mybir as mybir
    from concourse.masks import make_identity
    from concourse.tile import TileContext

    f16 = mybir.dt.float16
    f32 = mybir.dt.float32

    nc = bacc.Bacc(None, target_bir_lowering=False, debug=False)
    fin = nc.dram_tensor("fin", [IMGS_PER_CORE, 128, F_FREE], f16,
                         kind="ExternalInput")
    win = nc.dram_tensor("win", [IMGS_PER_CORE, K, 128, WG_FREE], f16,
                         kind="ExternalInput")
    oout = nc.dram_tensor("oout", [IMGS_PER_CORE, 128, O_FREE], f16,
                          kind="ExternalOutput")

    with TileContext(nc) as tc:
        with (
            tc.tile_pool(name="const", bufs=1) as cpool,
            tc.tile_pool(name="fpool", bufs=2) as fpool,
            tc.tile_pool(name="wpool", bufs=3) as wpool,
            tc.tile_pool(name="wtap", bufs=6) as wtpool,
            tc.tile_pool(name="tpool", bufs=3) as tpool,
            tc.tile_pool(name="ttap", bufs=6) as ttpool,
            tc.tile_pool(name="opool", bufs=2) as opool,
            tc.tile_pool(name="psum", bufs=2, space="PSUM") as ppool,
        ):
            ident = cpool.tile([128, 128], f16)
            make_identity(nc, ident[:])
            zmov = cpool.tile([128, 512], f16)
            nc.scalar.memzero(zmov[:])

            pss = []
            for img in range(IMGS_PER_CORE):
                f_t = fpool.tile([128, F_FREE], f16, tag="f")
                nc.scalar.dma_start(out=f_t[:], in_=fin[img])
                fv = f_t[:].rearrange("p (r y) -> p r y", r=FROWS, y=YC)

                ps = ppool.tile([128, O_FREE], f32, tag="ps")
                pss.append(ps)
                if img == 0:
                    # PE clock warmup while the first DMAs stream in.
                    for _ in range(N_WARMUP):
                        nc.tensor.matmul(ps[:, 0:512], ident[:], zmov[:],
                                         start=True, stop=True)

                def consume_tap(t, tmp_ap, off0):
                    # tmp_ap: [128, T_FREE] product tile of tap t; the
                    # moving slice for output row b starts at b*YC + j
                    # (ISA caps the moving free dim at 512).
                    j = t % K
                    for b in range(RPP):
                        mv = tmp_ap[:, b * YC + j: b * YC + j + W]
                        nc.tensor.matmul(
                            ps[:, b * W:(b + 1) * W], ident[:], mv,
                            start=(t == 0), stop=(t == K * K - 1))

                for tg in range(K):
                    per_tap = (img == 0 and tg == 0) or (
                        img == IMGS_PER_CORE - 1 and tg == K - 1)
                    if per_tap:
                        for k in range(K):
                            t = tg * K + k
                            w_t = wtpool.tile([128, T_FREE], f16)
                            nc.sync.dma_start(
                                out=w_t[:],
                                in_=win[img, tg][:, k * T_FREE:
                                                 (k + 1) * T_FREE])
                            tmp = ttpool.tile([128, T_FREE], f16)
                            nc.vector.tensor_mul(
                                out=tmp[:].rearrange(
                                    "p (r y) -> p r y", r=RPP, y=YC),
                                in0=w_t[:].rearrange(
                                    "p (r y) -> p r y", r=RPP, y=YC),
                                in1=fv[:, tg:tg + RPP, :])
                            consume_tap(t, tmp[:], 0)
                    else:
                        w_t = wpool.tile([128, WG_FREE], f16)
                        nc.sync.dma_start(out=w_t[:], in_=win[img, tg])
                        tmp = tpool.tile([128, WG_FREE], f16)
                        fbc = fv[:, tg:tg + RPP, :].unsqueeze(1).broadcast_to(
                            [128, K, RPP, YC])
                        nc.vector.tensor_mul(
                            out=tmp[:].rearrange(
                                "p (k r y) -> p k r y", k=K, r=RPP, y=YC),
                            in0=w_t[:].rearrange(
                                "p (k r y) -> p k r y", k=K, r=RPP, y=YC),
                            in1=fbc)
                        for k in range(K):
                            consume_tap(tg * K + k,
                                        tmp[:, k * T_FREE:(k + 1) * T_FREE],
                                        k * T_FREE)

                o_t = opool.tile([128, O_FREE], f16, tag="o")
                nc.scalar.activation(
                    out=o_t[:], in_=ps[:],
                    func=mybir.ActivationFunctionType.Copy)
                nc.scalar.dma_start(out=oout[img], in_=o_t[:])
    nc.finalize()
    return nc


def _host_prep(frames, core):
    """Build per-core in_maps. frames [4,4,1,512,512] f32, core [4,4,25,1,512,512]."""
    G = NCORES * IMGS_PER_CORE  # 16
    F = np.ascontiguousarray(frames.reshape(G, H, W))
    Wc = core.reshape(G, K * K, H, W)

    # frames: pad rows 2/2, cols 2/2 -> [G, 516, 516]; stored row rr at
    # partition p = image row 4p-2+rr = padded row 4p+rr; col y = image y-2.
    Fp = np.pad(F, ((0, 0), (2, 2), (2, 2))).astype(np.float16)
    rows = 4 * np.arange(128)[:, None] + np.arange(FROWS)[None, :]  # [128,8]
    fprep = Fp[:, rows, :]  # [G, 128, 8, 516]

    # weights: shift tap (i,j) into the y-grid at y = x + j, zero elsewhere.
    w16 = Wc.astype(np.float16)
    wsh = np.zeros((G, K * K, H, YC), np.float16)
    for t in range(K * K):
        j = t % K
        wsh[:, t, :, j:j + W] = w16[:, t]
    # [g, t, 4p+r, y] -> [g, tg, p, k, r, y]
    wprep = np.ascontiguousarray(
        wsh.reshape(G, K, K, 128, RPP, YC).transpose(0, 1, 3, 2, 4, 5))

    in_maps = []
    for c in range(NCORES):
        g0 = c * IMGS_PER_CORE
        in_maps.append({
            "fin": np.ascontiguousarray(
                fprep[g0:g0 + IMGS_PER_CORE].reshape(
                    IMGS_PER_CORE, 128, F_FREE)),
            "win": np.ascontiguousarray(
                wprep[g0:g0 + IMGS_PER_CORE].reshape(
                    IMGS_PER_CORE, K, 128, WG_FREE)),
        })
    return in_maps


def kernel(frames, core, bias):
    global last_results
    from concourse.bass_utils import run_bass_kernel_spmd

    frames = np.asarray(frames, dtype=np.float32)
    core = np.asarray(core, dtype=np.float32)

    if "nc" not in _compiled:
        _compiled["nc"] = _build_nc()
    nc = _compiled["nc"]

    in_maps = _host_prep(frames, core)
    trace = os.environ.get("KC_TRACE") == "1"
    tmpdir = os.environ.get("KC_TRACE_DIR") or None
    if tmpdir:
        os.makedirs(tmpdir, exist_ok=True)
    res = run_bass_kernel_spmd(nc, in_maps, list(range(NCORES)), trace=trace,
                               tmpdir=tmpdir)
    last_results = res

    G = NCORES * IMGS_PER_CORE
    out = np.empty((G, H, W), np.float32)
    for c in range(NCORES):
        o = res.results[c]["oout"]  # [2, 128, 2048] f16
        for img in range(IMGS_PER_CORE):
            out[c * IMGS_PER_CORE + img] = (
                o[img].astype(np.float32).reshape(H, W))
    return out.reshape(4, 4, H, W)


# revision 11
# speedup vs baseline: 1.0735x; 1.0735x over previous
"""Per-pixel adaptive 5x5 conv (KPN) for Trainium2, 8-core data parallel.

out[g,h,w] = sum_{i,j} core[g,5i+j,h,w] * frames_pad[g,h+i-2,w+j-2]
with g = flattened (B,N) = 16 image planes; 2 planes per NeuronCore.

v3 design — PE-accumulate, DVE-multiply, no GpSimd compute:
  Row layout: partition p owns output rows 4p..4p+3; fin stores the 8-row
  halo (4p-2..4p+5) x 516 padded cols per partition (x2 replication only).
  Weights are host-shifted into a 516-wide y-grid per tap (w at y=x+j) so
  every DVE read starts 4B-aligned. All 5 taps of group tg share i=tg, so
  ONE fp16 tensor_mul per group computes all 5 products (frames broadcast
  along the tap dim with a stride-0 AP):
     q[p,k,r,y] = w[p,k,r,y] * f[p,r+tg,y]
  The IDLE TensorEngine sums the 25 taps per output row r into PSUM (fp32)
  as identity-stationary matmuls, moving slice [2,512] at offset r*516+j
  (PE needs no alignment). First group of img0 runs per-tap (small DMAs +
  muls) to cut the startup ramp; warmup matmuls during the ramp release
  the PE HAM clock throttle. ACT evacuates PSUM with the fp32->fp16 cast.
  fin/out ride the second HWDGE ring (nc.scalar), weights on nc.sync.
  GpSimd does nothing: its SBUF port contends with DVE tensor_tensor.
"""

import os
import sys

import numpy as np

for _p in ("/opt/trn_rl_repo",):
    if _p not in sys.path and os.path.isdir(_p):
        sys.path.insert(0, _p)

def _patch_ldw_opt():
    """Enable walrus ldw-opt so the 200+ identical identity LDWEIGHTS
    dedup instead of serializing with every matmul."""
    import concourse.bass_utils as _bu
    if getattr(_bu, "_ldw_patched", False):
        return

    # NOTE: --enable-ldw-opt=true crashes walrus codegen
    # (visitInstLdweights); leave the command untouched.
    _bu._ldw_patched = True


K = 5
NCORES = 8
IMGS_PER_CORE = 2
H = W = 512
RPP = 4            # output rows per partition
FROWS = 8          # stored halo rows per partition
YC = 516           # padded column grid
F_FREE = FROWS * YC          # 4128
T_FREE = RPP * YC            # 2064 (one tap)
WG_FREE = K * T_FREE         # 10320 (one 5-tap group)
O_FREE = RPP * W             # 2048
N_WARMUP = 12

_compiled = {}
last_results = None  # BassKernelResults of the most recent run (for test.py)


def _build_nc():
    import concourse.bacc as bacc
    import concourse.mybir as mybir
    from concourse.masks import make_identity
    from concourse.tile import TileContext

    f16 = mybir.dt.float16
    f32 = mybir.dt.float32

    nc = bacc.Bacc(None, target_bir_lowering=False, debug=False)
    fin = nc.dram_tensor("fin", [IMGS_PER_CORE, 128, F_FREE], f16,
                         kind="ExternalInput")
    win = nc.dram_tensor("win", [IMGS_PER_CORE, K, 128, WG_FREE], f16,
                         kind="ExternalInput")
    oout = nc.dram_tensor("oout", [IMGS_PER_CORE, 128, O_FREE], f16,
                          kind="ExternalOutput")

    with TileContext(nc) as tc:
        with (
            tc.tile_pool(name="const", bufs=1) as cpool,
            tc.tile_pool(name="fpool", bufs=2) as fpool,
            tc.tile_pool(name="wpool", bufs=3) as wpool,
            tc.tile_pool(name="wtap", bufs=6) as wtpool,
            tc.tile_pool(name="ttap", bufs=6) as ttpool,
            tc.tile_pool(name="opool", bufs=2) as opool,
            tc.tile_pool(name="psum", bufs=2, space="PSUM") as ppool,
        ):
            ident = cpool.tile([128, 128], f16)
            make_identity(nc, ident[:])
            zmov = cpool.tile([128, 512], f16)
            nc.scalar.memzero(zmov[:])

            pss = []
            for img in range(IMGS_PER_CORE):
                f_t = fpool.tile([128, F_FREE], f16, tag="f")
                nc.scalar.dma_start(out=f_t[:], in_=fin[img])
                fv = f_t[:].rearrange("p (r y) -> p r y", r=FROWS, y=YC)

                ps = ppool.tile([128, O_FREE], f32, tag="ps")
                pss.append(ps)
                if img == 0:
                    # PE clock warmup while the first DMAs stream in.
                    for _ in range(N_WARMUP):
                        nc.tensor.matmul(ps[:, 0:512], ident[:], zmov[:],
                                         start=True, stop=True)

                def consume_tap(t, tmp_ap, off0):
                    # tmp_ap: [128, T_FREE] product tile of tap t; the
                    # moving slice for output row b starts at b*YC + j
                    # (ISA caps the moving free dim at 512).
                    j = t % K
                    for b in range(RPP):
                        mv = tmp_ap[:, b * YC + j: b * YC + j + W]
                        nc.tensor.matmul(
                            ps[:, b * W:(b + 1) * W], ident[:], mv,
                            start=(t == 0), stop=(t == K * K - 1))

                for tg in range(K):
                    split_dma = img == 0 and tg == 0
                    if split_dma:
                        # startup ramp: per-tap DMAs so the first mul can
                        # begin after ~0.5MB instead of ~2.6MB.
                        wts = []
                        for k in range(K):
                            w_t = wtpool.tile([128, T_FREE], f16)
                            nc.sync.dma_start(
                                out=w_t[:],
                                in_=win[img, tg][:, k * T_FREE:
                                                 (k + 1) * T_FREE])
                            wts.append(w_t[:].rearrange(
                                "p (r y) -> p r y", r=RPP, y=YC))
                    else:
                        w_t = wpool.tile([128, WG_FREE], f16)
                        nc.sync.dma_start(out=w_t[:], in_=win[img, tg])
                        wv = w_t[:].rearrange("p (k r y) -> p k r y",
                                              k=K, r=RPP, y=YC)
                        wts = [wv[:, k] for k in range(K)]
                    for k in range(K):
                        t = tg * K + k
                        tmp = ttpool.tile([128, T_FREE], f16)
                        nc.vector.tensor_mul(
                            out=tmp[:].rearrange(
                                "p (r y) -> p r y", r=RPP, y=YC),
                            in0=wts[k],
                            in1=fv[:, tg:tg + RPP, :])
                        consume_tap(t, tmp[:], 0)

                o_t = opool.tile([128, O_FREE], f16, tag="o")
                nc.scalar.activation(
                    out=o_t[:], in_=ps[:],
                    func=mybir.ActivationFunctionType.Copy)
                nc.scalar.dma_start(out=oout[img], in_=o_t[:])
    nc.finalize()
    return nc


def _host_prep(frames, core):
    """Build per-core in_maps. frames [4,4,1,512,512] f32, core [4,4,25,1,512,512]."""
    G = NCORES * IMGS_PER_CORE  # 16
    F = np.ascontiguousarray(frames.reshape(G, H, W))
    Wc = core.reshape(G, K * K, H, W)

    # frames: pad rows 2/2, cols 2/2 -> [G, 516, 516]; stored row rr at
    # partition p = image row 4p-2+rr = padded row 4p+rr; col y = image y-2.
    Fp = np.pad(F, ((0, 0), (2, 2), (2, 2))).astype(np.float16)
    rows = 4 * np.arange(128)[:, None] + np.arange(FROWS)[None, :]  # [128,8]
    fprep = Fp[:, rows, :]  # [G, 128, 8, 516]

    # weights: shift tap (i,j) into the y-grid at y = x + j, zero elsewhere.
    w16 = Wc.astype(np.float16)
    wsh = np.zeros((G, K * K, H, YC), np.float16)
    for t in range(K * K):
        j = t % K
        wsh[:, t, :, j:j + W] = w16[:, t]
    # [g, t, 4p+r, y] -> [g, tg, p, k, r, y]
    wprep = np.ascontiguousarray(
        wsh.reshape(G, K, K, 128, RPP, YC).transpose(0, 1, 3, 2, 4, 5))

    in_maps = []
    for c in range(NCORES):
        g0 = c * IMGS_PER_CORE
        in_maps.append({
            "fin": np.ascontiguousarray(
                fprep[g0:g0 + IMGS_PER_CORE].reshape(
                    IMGS_PER_CORE, 128, F_FREE)),
            "win": np.ascontiguousarray(
                wprep[g0:g0 + IMGS_PER_CORE].reshape(
                    IMGS_PER_CORE, K, 128, WG_FREE)),
        })
    return in_maps


def kernel(frames, core, bias):
    global last_results
    from concourse.bass_utils import run_bass_kernel_spmd

    frames = np.asarray(frames, dtype=np.float32)
    core = np.asarray(core, dtype=np.float32)

    _patch_ldw_opt()
    if "nc" not in _compiled:
        _compiled["nc"] = _build_nc()
    nc = _compiled["nc"]

    in_maps = _host_prep(frames, core)
    trace = os.environ.get("KC_TRACE") == "1"
    tmpdir = os.environ.get("KC_TRACE_DIR") or None
    if tmpdir:
        os.makedirs(tmpdir, exist_ok=True)
    res = run_bass_kernel_spmd(nc, in_maps, list(range(NCORES)), trace=trace,
                               tmpdir=tmpdir)
    last_results = res

    G = NCORES * IMGS_PER_CORE
    out = np.empty((G, H, W), np.float32)
    for c in range(NCORES):
        o = res.results[c]["oout"]  # [2, 128, 2048] f16
        for img in range(IMGS_PER_CORE):
            out[c * IMGS_PER_CORE + img] = (
                o[img].astype(np.float32).reshape(H, W))
    return out.reshape(4, 4, H, W)
